# revision 4
# baseline (speedup 1.0000x reference)
"""Trainium2 Bass kernel for nn_LogLinearAttention.

Math: the reference computes
    q = x@Wq.T+bq ; v = x@Wv.T+bv ; r = x@Wr.T+br
    scores = q @ v.T ; attn = softmax(scores, axis=1)   # over the QUERY axis
    emb = attn.T-contract r ; pooled = emb.sum(axis=1)  # over the SAME axis
    out = sigmoid(pooled @ Wl.T + bl)

Because softmax normalizes over axis 1 and pooled sums over axis 1,
sum_s attn[s, t] == 1 for every t, so
    pooled[b] = sum_t r[b, t, :] = (sum_t x[b, t, :]) @ Wr.T + S*br
and the q/v projections and the S x S attention cancel exactly:
    out[b] = sigmoid( xsum[b] . (Wl@Wr) + S*(br . Wl) + bl )

The kernel therefore only needs a sequence-sum of x (the only large input,
32MB) plus tiny weight contractions. Data-parallel over batch: core b
handles x[b] (4MB), weights replicated.

Per-core device program:
  - x[b] arrives as [4, 128, 4, 512]: four 1MB DMAs, each a contiguous
    slab with 8KB/partition runs (near peak HBM BW).
  - xsum[1,512] accumulates in PSUM via matmuls with a ones[128,1]
    stationary operand (fp32r: 1 cycle/row at N=512).
  - w = Wl@Wr from one 1MB Wr DMA, contracting the natural row-major
    partition layout (no transposes anywhere).
  - logit = dot(xsum, w) + S*dot(br, Wl) + bl on 1 partition; sigmoid on
    the scalar engine; DMA the [1,1] result out.
"""

import numpy as np

B, S, D = 8, 2048, 512
P = 128
NQ = 4  # x DMA chunks per core (1MB each)
KS = 4  # rows per partition per chunk slice
JW = 4  # Wr/Wl/br rows per partition

_CACHE = {}


def _build():
    import concourse.bacc as bacc
    import concourse.mybir as mybir
    import concourse.tile as tile

    f32 = mybir.dt.float32
    bf16 = mybir.dt.bfloat16

    nc = bacc.Bacc(
        "TRN2",
        target_bir_lowering=False,
        debug=False,
        enable_asserts=False,
        num_devices=B,
    )
    x_d = nc.dram_tensor("x", [NQ, P, KS, D], f32, kind="ExternalInput").ap()
    wr_d = nc.dram_tensor("wr", [P, JW, D], f32, kind="ExternalInput").ap()
    wl_d = nc.dram_tensor("wl", [P, JW], f32, kind="ExternalInput").ap()
    br_d = nc.dram_tensor("br", [P, JW], f32, kind="ExternalInput").ap()
    bl_d = nc.dram_tensor("bl", [1, 1], f32, kind="ExternalInput").ap()
    out_d = nc.dram_tensor("out", [1, 1], f32, kind="ExternalOutput").ap()

    with tile.TileContext(nc) as tc:
        with (
            tc.tile_pool(name="xp", bufs=3) as xp,
            tc.tile_pool(name="sg", bufs=1) as sg,
            tc.tile_pool(name="ps", bufs=1, space="PSUM") as ps,
        ):
            ones = sg.tile([P, 1], f32, tag="ones")
            nc.vector.memset(ones, 1.0)
            ones_bf = sg.tile([P, 1], bf16, tag="ones_bf")
            nc.vector.memset(ones_bf, 1.0)
            wlt = sg.tile([P, JW], f32, tag="wlt")
            nc.sync.dma_start(wlt, wl_d)
            wlt_bf = sg.tile([P, JW], bf16, tag="wlt_bf")
            nc.any.tensor_copy(out=wlt_bf, in_=wlt)
            brt = sg.tile([P, JW], f32, tag="brt")
            nc.sync.dma_start(brt, br_d)
            blt = sg.tile([1, 1], f32, tag="blt")
            nc.sync.dma_start(blt, bl_d)

            # xsum[1, D] = sum_t x[t, :], accumulated over 16 matmuls.
            # The x stream is cast fp32->bf16 in-flight (SWDGE cast DMA):
            # HBM reads are unchanged, matmuls run at full bf16 rate, and
            # PSUM accumulates in fp32.
            xs_ps = ps.tile([1, D], f32, tag="xs")
            for q in range(NQ):
                xt = xp.tile([P, KS, D], bf16, tag="xt")
                nc.gpsimd.dma_start(xt, x_d[q])
                for k in range(KS):
                    nc.tensor.matmul(
                        xs_ps,
                        ones_bf,
                        xt[:, k, :],
                        start=(q == 0 and k == 0),
                        stop=(q == NQ - 1 and k == KS - 1),
                    )

            # w[1, D] = Wl @ Wr (contract over output-channel i)
            wt = sg.tile([P, JW, D], bf16, tag="wt")
            nc.gpsimd.dma_start(wt, wr_d)
            w_ps = ps.tile([1, D], f32, tag="w")
            for j in range(JW):
                nc.tensor.matmul(
                    w_ps,
                    wlt_bf[:, j : j + 1],
                    wt[:, j, :],
                    start=(j == 0),
                    stop=(j == JW - 1),
                )

            # logit = dot(xsum, w) + S * dot(br, Wl) + bl ; out = sigmoid
            w_sb = sg.tile([1, D], f32, tag="w_sb")
            nc.any.tensor_copy(out=w_sb, in_=w_ps)
            prod = sg.tile([1, D], f32, tag="prod")
            nc.vector.tensor_mul(out=prod, in0=xs_ps, in1=w_sb)
            t1 = sg.tile([1, 1], f32, tag="t1")
            nc.vector.reduce_sum(t1, prod, axis=mybir.AxisListType.X)

            prod2 = sg.tile([P, JW], f32, tag="prod2")
            nc.vector.tensor_mul(out=prod2, in0=brt, in1=wlt)
            red2 = sg.tile([P, 1], f32, tag="red2")
            nc.vector.reduce_sum(red2, prod2, axis=mybir.AxisListType.X)
            c_ps = ps.tile([1, 1], f32, tag="c")
            nc.tensor.matmul(c_ps, red2, ones, start=True, stop=True)

            t2 = sg.tile([1, 1], f32, tag="t2")
            nc.vector.tensor_add(out=t2, in0=t1, in1=blt)
            fin = sg.tile([1, 1], f32, tag="fin")
            nc.scalar.activation(
                fin,
                c_ps,
                mybir.ActivationFunctionType.Sigmoid,
                bias=t2,
                scale=float(S),
            )
            nc.sync.dma_start(out_d, fin)

    nc.compile()
    return nc


def _in_maps(inputs):
    x = np.ascontiguousarray(np.asarray(inputs["x"], dtype=np.float32))
    Wr = np.ascontiguousarray(np.asarray(inputs["Wr"], dtype=np.float32))
    br = np.asarray(inputs["br"], dtype=np.float32)
    Wl = np.asarray(inputs["Wl"], dtype=np.float32)
    bl = np.asarray(inputs["bl"], dtype=np.float32)

    wr_h = Wr.reshape(P, JW, D)
    wl_h = np.ascontiguousarray(Wl.reshape(P, JW))
    br_h = np.ascontiguousarray(br.reshape(P, JW))
    bl_h = bl.reshape(1, 1)
    return [
        {
            "x": x[b].reshape(NQ, P, KS, D),
            "wr": wr_h,
            "wl": wl_h,
            "br": br_h,
            "bl": bl_h,
        }
        for b in range(B)
    ]


def get_nc():
    if "nc" not in _CACHE:
        _CACHE["nc"] = _build()
    return _CACHE["nc"]


def kernel(**inputs) -> np.ndarray:
    from concourse.bass_utils import run_bass_kernel_spmd

    nc = get_nc()
    res = run_bass_kernel_spmd(nc, _in_maps(inputs), list(range(B)))
    out = np.stack([res.results[b]["out"].reshape(()) for b in range(B)])
    return out.reshape(B, 1).astype(np.float32)
